# revision 32
# baseline (speedup 1.0000x reference)
"""Trainium2 Bass kernel for nn_AutoShot (histogram binning + windowed similarity + FC).

Sharding: data-parallel over B*T = 400 frames -> 8 cores x 50 frames.

Phase A (heavy): per-core color histograms [50, 512].
  bin = hi5 * 16 + lo4 with hi5 = ((R>>3)&28)|(G>>6), lo4 = ((G>>2)&8)|(B>>5).
  Instead of one-hot factors, each side uses an arbitrary INVERTIBLE basis:
    M[i, j] = sum_px Abasis_i(hi_px) * Bbasis_j(lo_px)   (PE, PSUM-accumulated)
    counts  = Ainv @ M @ Binv^T                          (host, float64, exact)
  Basis columns are chosen by per-engine cost so all three elementwise
  engines run flat out in parallel:
    - free: ones (Pool memset), a2=(R>>3)&28, b2=G>>6, c2=(G>>2)&8, d2=B>>5
      (prep byproducts cast straight into the operand tiles)
    - DVE: is_equal (tensor_scalar, 4x perf mode, ~265ns/col)
    - ACT: exact sign staircases sgn(x - (k+.5)) (1 op/col, ~842ns)
    - Pool: is_equal tensor_scalar (~1186ns/col); one column is split
      DVE/Pool for load balance
  All products are small integers; fp32 PSUM sums stay < 2^24 -> exact.
  Channel planes are uint16 so every DVE op runs in the 4x perf mode.
  Batch schedule [1]+[2]*24+[1] frames shrinks pipeline fill/drain tails;
  finished frames stream to DRAM incrementally.

Phase B (light): per-core sim = xh @ xs^T (xs = zero-padded +-50 frame
  context) in float32r, diagonal window extract via a stride-164 read over
  stride-163 rows in a DRAM scratch (addr 164*t + l = sim[t, t+l]), PE
  transpose, FC matmul (W [128,101]), output written [P, NF]-major to keep
  the out-DMA contiguous.

Host: slices inputs, reconstructs counts (tiny 32x32/16x16 inverses),
  L2-normalizes, applies bias + ReLU, reassembles the [4,100,128] output.
"""

import sys

for _p in ("/opt/trn_rl_repo", "/root/.axon_site/_ro/trn_rl_repo"):
    if _p not in sys.path:
        sys.path.append(_p)

import numpy as np

from concourse import bass, bacc, mybir
import concourse.tile as tile
from concourse.bass_utils import run_bass_kernel_spmd
from concourse.masks import make_identity

P = 128
NPIX = 224 * 224        # 50176 pixels per frame plane
FPP = NPIX // P         # 392 pixels per partition
NF = 50                 # frames per core
V1, V2 = 32, 16         # 512 = 32 * 16 bin split
LW = 101
NCORES = 8
F32 = mybir.dt.float32
F32R = mybir.dt.float32r
U16 = mybir.dt.uint16
BF16 = mybir.dt.bfloat16
OP = mybir.AluOpType
AF = mybir.ActivationFunctionType

# ---------------------------------------------------------------------------
# Basis column specs.  Each entry: ("ones"|"a2"|"b2"|"c2"|"d2") for free
# columns, ("dve", v) / ("pool", v) for is_equal(v), ("act", thr) for
# sgn(x - thr).  The same spec drives the device builder and the host-side
# inverse (evaluated in numpy below).
USE_POOL = True
USE_ACT = True


def _mk_specs():
    # A side: free rows resolve points {0,3,4}; deltas at [1,2]+[5..23]
    # (17 DVE + 4 Pool); ACT staircases 23.5..30.5 resolve 24..31.
    a_spec = [("ones",), ("a2",), ("b2",)]
    if USE_ACT:
        a_delta = [1, 2] + list(range(5, 24))
        act_lo = 23
    else:
        a_delta = [1, 2] + list(range(5, 32))
        act_lo = 31
    n_pool_a = 4 if USE_POOL else 0
    n_dve_a = len(a_delta) - n_pool_a
    for i, p in enumerate(a_delta[:n_dve_a]):
        kind = "dvepool" if (USE_POOL and i == 0) else "dve"
        a_spec.append((kind, float(p)))
    for p in a_delta[len(a_delta) - n_pool_a:]:
        a_spec.append(("pool", float(p)))
    for t in range(act_lo, 31):
        a_spec.append(("act", t + 0.5))
    # B side: free rows resolve {0,7,8}; deltas at [1..6, 9..12]
    # (8 DVE + 2 Pool); ACT staircases 12.5/13.5/14.5 resolve 13..15.
    b_spec = [("ones",), ("c2",), ("d2",)]
    b_delta = [1, 2, 3, 4, 5, 6, 9, 10, 11, 12]
    n_pool_b = 2 if USE_POOL else 0
    for p in b_delta[: len(b_delta) - n_pool_b]:
        b_spec.append(("dve", float(p)))
    for p in b_delta[len(b_delta) - n_pool_b:]:
        b_spec.append(("pool", float(p)))
    if USE_ACT:
        for t in range(12, 15):
            b_spec.append(("act", t + 0.5))
    else:
        for p in range(13, 16):
            b_spec.append(("dve", float(p)))
    assert len(a_spec) == V1 and len(b_spec) == V2
    return a_spec, b_spec


A_SPEC, B_SPEC = _mk_specs()


def _basis_matrix(spec, nvals, a_side):
    """Rows = basis columns, cols = alphabet values; float64."""
    v = np.arange(nvals, dtype=np.float64)
    rows = []
    for s in spec:
        kind = s[0]
        if kind == "ones":
            rows.append(np.ones(nvals))
        elif kind == "a2":
            rows.append(np.float64(np.arange(nvals) & 28))
        elif kind == "b2":
            rows.append(np.float64(np.arange(nvals) & 3))
        elif kind == "c2":
            rows.append(np.float64(np.arange(nvals) & 8))
        elif kind == "d2":
            rows.append(np.float64(np.arange(nvals) & 7))
        elif kind in ("dve", "pool", "dvepool"):
            rows.append((v == s[1]).astype(np.float64))
        elif kind == "act":
            rows.append(np.sign(v - s[1]))
        else:
            raise ValueError(kind)
    return np.stack(rows)


ABASIS = _basis_matrix(A_SPEC, V1, True)
BBASIS = _basis_matrix(B_SPEC, V2, False)
AINV = np.linalg.inv(ABASIS)
BINV = np.linalg.inv(BBASIS)
assert np.linalg.cond(ABASIS) < 1e4 and np.linalg.cond(BBASIS) < 1e4


def build_hist_nc():
    nc = bacc.Bacc("TRN2")
    fr = nc.dram_tensor("fr", [3, NF, NPIX], U16, kind="ExternalInput")
    hist = nc.dram_tensor("hist", [NF, 512], F32, kind="ExternalOutput")
    G = 2                # frames per DVE batch
    FD = G * FPP         # 784 free-dim elements per DVE op

    act_bias = {}        # threshold -> AP [P,1] holding -thr

    def emit_col(tile_, idx, fd, src_tile, s):
        kind = s[0]
        lo = idx * fd
        if kind == "dvepool":
            h = (fd // 2) // 4 * 4
            nc.vector.tensor_scalar(
                out=tile_[:, lo:lo + h], in0=src_tile[:, 0:h], scalar1=s[1],
                scalar2=None, op0=OP.is_equal)
            nc.gpsimd.tensor_scalar(
                out=tile_[:, lo + h:lo + fd], in0=src_tile[:, h:fd],
                scalar1=s[1], scalar2=None, op0=OP.is_equal)
        elif kind == "dve":
            nc.vector.tensor_scalar(
                out=tile_[:, lo:lo + fd], in0=src_tile[:, 0:fd], scalar1=s[1],
                scalar2=None, op0=OP.is_equal)
        elif kind == "pool":
            nc.gpsimd.tensor_scalar(
                out=tile_[:, lo:lo + fd], in0=src_tile[:, 0:fd], scalar1=s[1],
                scalar2=None, op0=OP.is_equal)
        elif kind == "act":
            nc.scalar.activation(out=tile_[:, lo:lo + fd],
                                 in_=src_tile[:, 0:fd], func=AF.Sign,
                                 bias=act_bias[s[1]])
        elif kind == "ones":
            nc.gpsimd.memset(tile_[:, lo:lo + fd], 1.0)
        else:
            raise ValueError(kind)

    with tile.TileContext(nc) as tc:
        with (
            tc.tile_pool(name="io", bufs=3) as io,
            tc.tile_pool(name="mid", bufs=2) as mid,
            tc.tile_pool(name="oh", bufs=2) as oh,
            tc.tile_pool(name="cst", bufs=1) as cst,
            tc.tile_pool(name="ps", bufs=8, space="PSUM") as ps,
        ):
            osb = cst.tile([V1, NF * V2], F32)  # [32, 800] result staging

            thrs = sorted({s[1] for s in A_SPEC + B_SPEC if s[0] == "act"})
            bias_sb = cst.tile([P, max(len(thrs), 1)], F32)
            for i, thr in enumerate(thrs):
                nc.vector.memset(bias_sb[:, i:i + 1], -float(thr))
                act_bias[thr] = bias_sb[:, i:i + 1]

            # batch schedule: small first/last batches shrink pipeline
            # fill (PE idle) and drain (featurize engines idle) tails
            batches = []
            t = 0
            for gsz in [1] + [2] * ((NF - 2) // 2) + [1]:
                batches.append((t, gsz))
                t += gsz
            assert t == NF

            ia2 = A_SPEC.index(("a2",))
            ib2 = A_SPEC.index(("b2",))
            ic2 = B_SPEC.index(("c2",))
            id2 = B_SPEC.index(("d2",))

            build_hist_nc._sent = 0
            for t0, gsz in batches:
                fd = gsz * FPP
                r = io.tile([P, fd], U16, tag="ch_r")
                g = io.tile([P, fd], U16, tag="ch_g")
                b = io.tile([P, fd], U16, tag="ch_b")
                for ci, ch in ((0, r), (1, g), (2, b)):
                    nc.sync.dma_start(
                        out=ch[:].rearrange("p (q f) -> p q f", q=gsz),
                        in_=fr[ci, t0:t0 + gsz].rearrange(
                            "q (p f) -> p q f", p=P))

                A = oh.tile([P, V1 * fd], BF16, tag="A")
                B = oh.tile([P, V2 * fd], BF16, tag="B")

                def col(tile_, idx):
                    return tile_[:, idx * fd:(idx + 1) * fd]

                # prep: bitvec ops stay u16->u16, then numeric casts (4x)
                # write the byproducts into the operand tiles as bf16
                a2u = mid.tile([P, fd], U16, tag="a2u")
                nc.vector.tensor_scalar(
                    out=a2u[:], in0=r[:], scalar1=3, scalar2=28,
                    op0=OP.logical_shift_right, op1=OP.bitwise_and)
                b2u = mid.tile([P, fd], U16, tag="b2u")
                nc.vector.tensor_scalar(
                    out=b2u[:], in0=g[:], scalar1=6, scalar2=None,
                    op0=OP.logical_shift_right)
                nc.vector.tensor_copy(out=col(A, ia2), in_=a2u[:])
                nc.vector.tensor_copy(out=col(A, ib2), in_=b2u[:])
                hi_b = mid.tile([P, fd], BF16, tag="hi")
                nc.vector.tensor_tensor(
                    out=hi_b[:], in0=col(A, ia2), in1=col(A, ib2), op=OP.add)
                c2u = mid.tile([P, fd], U16, tag="c2u")
                nc.vector.tensor_scalar(
                    out=c2u[:], in0=g[:], scalar1=2, scalar2=8,
                    op0=OP.logical_shift_right, op1=OP.bitwise_and)
                d2u = mid.tile([P, fd], U16, tag="d2u")
                nc.vector.tensor_scalar(
                    out=d2u[:], in0=b[:], scalar1=5, scalar2=None,
                    op0=OP.logical_shift_right)
                nc.vector.tensor_copy(out=col(B, ic2), in_=c2u[:])
                nc.vector.tensor_copy(out=col(B, id2), in_=d2u[:])
                lo_b = mid.tile([P, fd], BF16, tag="lo")
                nc.vector.tensor_tensor(
                    out=lo_b[:], in0=col(B, ic2), in1=col(B, id2), op=OP.add)

                # non-DVE columns first so ACT/Pool queues fill early
                order = sorted(
                    range(V1), key=lambda i: A_SPEC[i][0] in ("dve", "dvepool"))
                for i in order:
                    s = A_SPEC[i]
                    if s[0] in ("a2", "b2"):
                        continue
                    emit_col(A, i, fd, hi_b, s)
                order = sorted(
                    range(V2), key=lambda i: B_SPEC[i][0] in ("dve", "dvepool"))
                for i in order:
                    s = B_SPEC[i]
                    if s[0] in ("c2", "d2"):
                        continue
                    emit_col(B, i, fd, lo_b, s)

                # contract over pixels per frame: M[i, j] += A_qj^T @ B_qj
                Aq = A[:].rearrange("p (v q f) -> p q f v", v=V1, q=gsz)
                Bq = B[:].rearrange("p (v q f) -> p q f v", v=V2, q=gsz)
                for q in range(gsz):
                    hps = ps.tile([V1, V2], F32)
                    for j in range(FPP):
                        nc.tensor.matmul(
                            out=hps[:],
                            lhsT=Aq[:, q, j, :],
                            rhs=Bq[:, q, j, :],
                            start=(j == 0), stop=(j == FPP - 1))
                    t = t0 + q
                    nc.scalar.activation(
                        out=osb[:, t * V2:(t + 1) * V2], in_=hps[:],
                        func=AF.Copy)

                # stream finished frames out incrementally so the final
                # DMA after the last batch is small
                done = t0 + gsz
                if done - getattr(build_hist_nc, "_sent", 0) >= 8 or done == NF:
                    lo_t = build_hist_nc._sent
                    nc.sync.dma_start(
                        out=hist[lo_t:done].rearrange(
                            "t (u w) -> u t w", u=V1),
                        in_=osb[:, lo_t * V2:done * V2].rearrange(
                            "u (t w) -> u t w", w=V2))
                    build_hist_nc._sent = done
    nc.compile()
    return nc


def build_fc_nc():
    """sim2 = xh @ xs^T [50,150]; win[t,l] = sim2[t, t+l]; out = relu(win@W^T + b)."""
    nc = bacc.Bacc("TRN2")
    # columns 0:50 = x_half^T, 50:200 = padded-context^T (one DMA -> one sem wait)
    xallT = nc.dram_tensor("xallT", [512, 200], F32R, kind="ExternalInput")
    wT = nc.dram_tensor("wT", [LW, P], F32R, kind="ExternalInput")
    out = nc.dram_tensor("out", [P, NF], F32, kind="ExternalOutput")
    # rows written at stride 163 (sim2[t] at 163*t), diagonal read back at
    # stride 164: addr 164*t + l = 163*t + (t+l) = sim2[t, t+l]  (no overlap)
    scratch = nc.dram_tensor("scratch", [NF * 164], F32, kind="Internal")

    with tile.TileContext(nc) as tc:
        with (
            tc.tile_pool(name="sb", bufs=1) as sb,
            tc.tile_pool(name="ps", bufs=1, space="PSUM") as ps,
        ):
            xa_sb = sb.tile([P, 4 * 200], F32R)
            nc.sync.dma_start(
                out=xa_sb[:].rearrange("p (a t) -> p a t", a=4),
                in_=xallT[:].rearrange("(a p) t -> p a t", p=P))
            wt_sb = sb.tile([LW, P], F32R)
            nc.sync.dma_start(out=wt_sb[:], in_=wT[:])

            sim_ps = ps.tile([NF, 150], F32)
            for a in range(4):
                nc.tensor.matmul(
                    out=sim_ps[:],
                    lhsT=xa_sb[:, a * 200:a * 200 + NF],
                    rhs=xa_sb[:, a * 200 + NF:(a + 1) * 200],
                    start=(a == 0), stop=(a == 3))
            sim_sb = sb.tile([NF, 150], F32)
            nc.vector.tensor_copy(out=sim_sb[:], in_=sim_ps[:])

            # row t of sim2 lands at flat offset 163*t
            nc.sync.dma_start(
                out=scratch[0:NF * 163].rearrange("(t c) -> t c", c=163)[:, 0:150],
                in_=sim_sb[:])
            # diagonal: win[t, l] = scratch[164*t + l] = sim2[t, t+l]
            win_sb = sb.tile([NF, LW], F32)
            nc.sync.dma_start(
                out=win_sb[:],
                in_=scratch[0:NF * 164].rearrange("(t c) -> t c", c=164)[:, 0:LW])

            # transpose win [50, 101] -> [101, 50] on the PE
            ident = sb.tile([NF, NF], F32)
            make_identity(nc, ident[:])
            win_ps = ps.tile([LW, NF], F32)
            nc.tensor.transpose(out=win_ps[:], in_=win_sb[:], identity=ident[:])
            win2 = sb.tile([LW, NF], F32R)
            nc.vector.tensor_copy(out=win2[:], in_=win_ps[:])

            fc_ps = ps.tile([P, NF], F32)
            nc.tensor.matmul(out=fc_ps[:], lhsT=wt_sb[:], rhs=win2[:],
                             start=True, stop=True)
            res = sb.tile([P, NF], F32)
            nc.vector.tensor_copy(out=res[:], in_=fc_ps[:])
            # bias + relu applied on host (tiny); avoids a 2-wait Activation
            nc.sync.dma_start(out=out[:], in_=res[:])
    nc.compile()
    return nc


_NC_CACHE = {}


def _get_nc(key, builder):
    if key not in _NC_CACHE:
        _NC_CACHE[key] = builder()
    return _NC_CACHE[key]


def kernel(frames, W, b):
    frames = np.asarray(frames, dtype=np.int32)
    W = np.asarray(W, dtype=np.float32)
    b = np.asarray(b, dtype=np.float32)
    Bn, _, T = frames.shape[:3]  # [4, 3, 100, 224, 224]

    nc_a = _get_nc("A", build_hist_nc)
    in_maps = []
    for c in range(NCORES):
        bi, h = c // 2, c % 2
        sl = frames[bi, :, h * NF:(h + 1) * NF].reshape(3, NF, NPIX)
        in_maps.append({"fr": np.ascontiguousarray(sl.astype(np.uint16))})
    res_a = run_bass_kernel_spmd(nc_a, in_maps, list(range(NCORES))).results

    counts = np.zeros((Bn, T, 512), np.float32)
    for c in range(NCORES):
        bi, h = c // 2, c % 2
        m = res_a[c]["hist"].astype(np.float64).reshape(NF, V1, V2)
        cts = np.einsum("ua,tuw,wb->tab", AINV.T, m, BINV.T)
        counts[bi, h * NF:(h + 1) * NF] = cts.reshape(NF, 512)
    xn = counts / np.linalg.norm(counts, axis=2, keepdims=True)

    nc_b = _get_nc("B", build_fc_nc)
    wT = np.ascontiguousarray(W.T)           # [101, 128]
    in_maps = []
    for c in range(NCORES):
        bi, h = c // 2, c % 2
        t0 = h * NF
        xall = np.zeros((200, 512), np.float32)
        xall[0:NF] = xn[bi, t0:t0 + NF]                  # x_half
        xall[NF + 50 - t0:NF + 50 - t0 + T] = xn[bi]     # xs[s'] = xn[s'+t0-50]
        in_maps.append({"xallT": np.ascontiguousarray(xall.T), "wT": wT})
    res_b = run_bass_kernel_spmd(nc_b, in_maps, list(range(NCORES))).results

    outp = np.zeros((Bn, T, P), np.float32)
    for c in range(NCORES):
        bi, h = c // 2, c % 2
        outp[bi, h * NF:(h + 1) * NF] = res_b[c]["out"].T
    outp = np.maximum(outp + b[None, None, :], 0.0)
    return outp


# revision 33
# speedup vs baseline: 1.0040x; 1.0040x over previous
"""Trainium2 Bass kernel for nn_AutoShot (histogram binning + windowed similarity + FC).

Sharding: data-parallel over B*T = 400 frames -> 8 cores x 50 frames.

Phase A (heavy): per-core color histograms [50, 512].
  bin = hi5 * 16 + lo4 with hi5 = ((R>>3)&28)|(G>>6), lo4 = ((G>>2)&8)|(B>>5).
  Instead of one-hot factors, each side uses an arbitrary INVERTIBLE basis:
    M[i, j] = sum_px Abasis_i(hi_px) * Bbasis_j(lo_px)   (PE, PSUM-accumulated)
    counts  = Ainv @ M @ Binv^T                          (host, float64, exact)
  Basis columns are chosen by per-engine cost so all three elementwise
  engines run flat out in parallel:
    - free: ones (Pool memset), a2=(R>>3)&28, b2=G>>6, c2=(G>>2)&8, d2=B>>5
      (prep byproducts cast straight into the operand tiles)
    - DVE: is_equal (tensor_scalar, 4x perf mode, ~265ns/col)
    - ACT: exact sign staircases sgn(x - (k+.5)) (1 op/col, ~842ns)
    - Pool: is_equal tensor_scalar (~1186ns/col); one column is split
      DVE/Pool for load balance
  All products are small integers; fp32 PSUM sums stay < 2^24 -> exact.
  Channel planes are uint16 so every DVE op runs in the 4x perf mode.
  Batch schedule [1]+[2]*24+[1] frames shrinks pipeline fill/drain tails;
  finished frames stream to DRAM incrementally.

Phase B (light): per-core sim = xh @ xs^T (xs = zero-padded +-50 frame
  context) in float32r, diagonal window extract via a stride-164 read over
  stride-163 rows in a DRAM scratch (addr 164*t + l = sim[t, t+l]), PE
  transpose, FC matmul (W [128,101]), output written [P, NF]-major to keep
  the out-DMA contiguous.

Host: slices inputs, reconstructs counts (tiny 32x32/16x16 inverses),
  L2-normalizes, applies bias + ReLU, reassembles the [4,100,128] output.
"""

import sys

for _p in ("/opt/trn_rl_repo", "/root/.axon_site/_ro/trn_rl_repo"):
    if _p not in sys.path:
        sys.path.append(_p)

import numpy as np
import ml_dtypes

from concourse import bass, bacc, mybir
import concourse.tile as tile
from concourse.bass_utils import run_bass_kernel_spmd
from concourse.masks import make_identity

P = 128
NPIX = 224 * 224        # 50176 pixels per frame plane
FPP = NPIX // P         # 392 pixels per partition
NF = 50                 # frames per core
V1, V2 = 32, 16         # 512 = 32 * 16 bin split
LW = 101
NCORES = 8
F32 = mybir.dt.float32
F32R = mybir.dt.float32r
U16 = mybir.dt.uint16
BF16 = mybir.dt.bfloat16
OP = mybir.AluOpType
AF = mybir.ActivationFunctionType

# ---------------------------------------------------------------------------
# Basis column specs.  Each entry: ("ones"|"a2"|"b2"|"c2"|"d2") for free
# columns, ("dve", v) / ("pool", v) for is_equal(v), ("act", thr) for
# sgn(x - thr).  The same spec drives the device builder and the host-side
# inverse (evaluated in numpy below).
USE_POOL = True
USE_ACT = True


def _mk_specs():
    # A side: free rows resolve points {0,3,4}; deltas at [1,2]+[5..23]
    # (17 DVE + 4 Pool); ACT staircases 23.5..30.5 resolve 24..31.
    a_spec = [("ones",), ("a2",), ("b2",)]
    if USE_ACT:
        a_delta = [1, 2] + list(range(5, 24))
        act_lo = 23
    else:
        a_delta = [1, 2] + list(range(5, 32))
        act_lo = 31
    n_pool_a = 4 if USE_POOL else 0
    n_dve_a = len(a_delta) - n_pool_a
    for i, p in enumerate(a_delta[:n_dve_a]):
        kind = "dvepool" if (USE_POOL and i == 0) else "dve"
        a_spec.append((kind, float(p)))
    for p in a_delta[len(a_delta) - n_pool_a:]:
        a_spec.append(("pool", float(p)))
    for t in range(act_lo, 31):
        a_spec.append(("act", t + 0.5))
    # B side: free rows resolve {0,7,8}; deltas at [1..6, 9..12]
    # (8 DVE + 2 Pool); ACT staircases 12.5/13.5/14.5 resolve 13..15.
    b_spec = [("ones",), ("c2",), ("d2",)]
    b_delta = [1, 2, 3, 4, 5, 6, 9, 10, 11, 12]
    n_pool_b = 2 if USE_POOL else 0
    for p in b_delta[: len(b_delta) - n_pool_b]:
        b_spec.append(("dve", float(p)))
    for p in b_delta[len(b_delta) - n_pool_b:]:
        b_spec.append(("pool", float(p)))
    if USE_ACT:
        for t in range(12, 15):
            b_spec.append(("act", t + 0.5))
    else:
        for p in range(13, 16):
            b_spec.append(("dve", float(p)))
    assert len(a_spec) == V1 and len(b_spec) == V2
    return a_spec, b_spec


A_SPEC, B_SPEC = _mk_specs()


def _basis_matrix(spec, nvals, a_side):
    """Rows = basis columns, cols = alphabet values; float64."""
    v = np.arange(nvals, dtype=np.float64)
    rows = []
    for s in spec:
        kind = s[0]
        if kind == "ones":
            rows.append(np.ones(nvals))
        elif kind == "a2":
            rows.append(np.float64(np.arange(nvals) & 28))
        elif kind == "b2":
            rows.append(np.float64(np.arange(nvals) & 3))
        elif kind == "c2":
            rows.append(np.float64(np.arange(nvals) & 8))
        elif kind == "d2":
            rows.append(np.float64(np.arange(nvals) & 7))
        elif kind in ("dve", "pool", "dvepool"):
            rows.append((v == s[1]).astype(np.float64))
        elif kind == "act":
            rows.append(np.sign(v - s[1]))
        else:
            raise ValueError(kind)
    return np.stack(rows)


ABASIS = _basis_matrix(A_SPEC, V1, True)
BBASIS = _basis_matrix(B_SPEC, V2, False)
AINV = np.linalg.inv(ABASIS)
BINV = np.linalg.inv(BBASIS)
assert np.linalg.cond(ABASIS) < 1e4 and np.linalg.cond(BBASIS) < 1e4


def build_hist_nc():
    nc = bacc.Bacc("TRN2")
    fr = nc.dram_tensor("fr", [3, NF, NPIX], U16, kind="ExternalInput")
    hist = nc.dram_tensor("hist", [NF, 512], F32, kind="ExternalOutput")
    G = 2                # frames per DVE batch
    FD = G * FPP         # 784 free-dim elements per DVE op

    act_bias = {}        # threshold -> AP [P,1] holding -thr

    def emit_col(tile_, idx, fd, src_tile, s):
        kind = s[0]
        lo = idx * fd
        if kind == "dvepool":
            h = (fd // 2) // 4 * 4
            nc.vector.tensor_scalar(
                out=tile_[:, lo:lo + h], in0=src_tile[:, 0:h], scalar1=s[1],
                scalar2=None, op0=OP.is_equal)
            nc.gpsimd.tensor_scalar(
                out=tile_[:, lo + h:lo + fd], in0=src_tile[:, h:fd],
                scalar1=s[1], scalar2=None, op0=OP.is_equal)
        elif kind == "dve":
            nc.vector.tensor_scalar(
                out=tile_[:, lo:lo + fd], in0=src_tile[:, 0:fd], scalar1=s[1],
                scalar2=None, op0=OP.is_equal)
        elif kind == "pool":
            nc.gpsimd.tensor_scalar(
                out=tile_[:, lo:lo + fd], in0=src_tile[:, 0:fd], scalar1=s[1],
                scalar2=None, op0=OP.is_equal)
        elif kind == "act":
            nc.scalar.activation(out=tile_[:, lo:lo + fd],
                                 in_=src_tile[:, 0:fd], func=AF.Sign,
                                 bias=act_bias[s[1]])
        elif kind == "ones":
            nc.gpsimd.memset(tile_[:, lo:lo + fd], 1.0)
        else:
            raise ValueError(kind)

    with tile.TileContext(nc) as tc:
        with (
            tc.tile_pool(name="io", bufs=3) as io,
            tc.tile_pool(name="mid", bufs=2) as mid,
            tc.tile_pool(name="oh", bufs=2) as oh,
            tc.tile_pool(name="cst", bufs=1) as cst,
            tc.tile_pool(name="ps", bufs=8, space="PSUM") as ps,
        ):
            osb = cst.tile([V1, NF * V2], F32)  # [32, 800] result staging

            thrs = sorted({s[1] for s in A_SPEC + B_SPEC if s[0] == "act"})
            bias_sb = cst.tile([P, max(len(thrs), 1)], F32)
            for i, thr in enumerate(thrs):
                nc.vector.memset(bias_sb[:, i:i + 1], -float(thr))
                act_bias[thr] = bias_sb[:, i:i + 1]

            # batch schedule: small first/last batches shrink pipeline
            # fill (PE idle) and drain (featurize engines idle) tails
            batches = []
            t = 0
            for gsz in [1] + [2] * ((NF - 2) // 2) + [1]:
                batches.append((t, gsz))
                t += gsz
            assert t == NF

            ia2 = A_SPEC.index(("a2",))
            ib2 = A_SPEC.index(("b2",))
            ic2 = B_SPEC.index(("c2",))
            id2 = B_SPEC.index(("d2",))

            build_hist_nc._sent = 0
            for t0, gsz in batches:
                fd = gsz * FPP
                r = io.tile([P, fd], U16, tag="ch_r")
                g = io.tile([P, fd], U16, tag="ch_g")
                b = io.tile([P, fd], U16, tag="ch_b")
                for ci, ch in ((0, r), (1, g), (2, b)):
                    nc.sync.dma_start(
                        out=ch[:].rearrange("p (q f) -> p q f", q=gsz),
                        in_=fr[ci, t0:t0 + gsz].rearrange(
                            "q (p f) -> p q f", p=P))

                A = oh.tile([P, V1 * fd], BF16, tag="A")
                B = oh.tile([P, V2 * fd], BF16, tag="B")

                def col(tile_, idx):
                    return tile_[:, idx * fd:(idx + 1) * fd]

                # prep: bitvec ops stay u16->u16, then numeric casts (4x)
                # write the byproducts into the operand tiles as bf16
                a2u = mid.tile([P, fd], U16, tag="a2u")
                nc.vector.tensor_scalar(
                    out=a2u[:], in0=r[:], scalar1=3, scalar2=28,
                    op0=OP.logical_shift_right, op1=OP.bitwise_and)
                b2u = mid.tile([P, fd], U16, tag="b2u")
                nc.vector.tensor_scalar(
                    out=b2u[:], in0=g[:], scalar1=6, scalar2=None,
                    op0=OP.logical_shift_right)
                nc.vector.tensor_copy(out=col(A, ia2), in_=a2u[:])
                nc.vector.tensor_copy(out=col(A, ib2), in_=b2u[:])
                hi_b = mid.tile([P, fd], BF16, tag="hi")
                nc.vector.tensor_tensor(
                    out=hi_b[:], in0=col(A, ia2), in1=col(A, ib2), op=OP.add)
                c2u = mid.tile([P, fd], U16, tag="c2u")
                nc.vector.tensor_scalar(
                    out=c2u[:], in0=g[:], scalar1=2, scalar2=8,
                    op0=OP.logical_shift_right, op1=OP.bitwise_and)
                d2u = mid.tile([P, fd], U16, tag="d2u")
                nc.vector.tensor_scalar(
                    out=d2u[:], in0=b[:], scalar1=5, scalar2=None,
                    op0=OP.logical_shift_right)
                nc.vector.tensor_copy(out=col(B, ic2), in_=c2u[:])
                nc.vector.tensor_copy(out=col(B, id2), in_=d2u[:])
                lo_b = mid.tile([P, fd], BF16, tag="lo")
                nc.vector.tensor_tensor(
                    out=lo_b[:], in0=col(B, ic2), in1=col(B, id2), op=OP.add)

                # non-DVE columns first so ACT/Pool queues fill early
                order = sorted(
                    range(V1), key=lambda i: A_SPEC[i][0] in ("dve", "dvepool"))
                for i in order:
                    s = A_SPEC[i]
                    if s[0] in ("a2", "b2"):
                        continue
                    emit_col(A, i, fd, hi_b, s)
                order = sorted(
                    range(V2), key=lambda i: B_SPEC[i][0] in ("dve", "dvepool"))
                for i in order:
                    s = B_SPEC[i]
                    if s[0] in ("c2", "d2"):
                        continue
                    emit_col(B, i, fd, lo_b, s)

                # contract over pixels per frame: M[i, j] += A_qj^T @ B_qj
                Aq = A[:].rearrange("p (v q f) -> p q f v", v=V1, q=gsz)
                Bq = B[:].rearrange("p (v q f) -> p q f v", v=V2, q=gsz)
                for q in range(gsz):
                    hps = ps.tile([V1, V2], F32)
                    for j in range(FPP):
                        nc.tensor.matmul(
                            out=hps[:],
                            lhsT=Aq[:, q, j, :],
                            rhs=Bq[:, q, j, :],
                            start=(j == 0), stop=(j == FPP - 1))
                    t = t0 + q
                    nc.scalar.activation(
                        out=osb[:, t * V2:(t + 1) * V2], in_=hps[:],
                        func=AF.Copy)

                # stream finished frames out incrementally so the final
                # DMA after the last batch is small
                done = t0 + gsz
                if done - getattr(build_hist_nc, "_sent", 0) >= 8 or done == NF:
                    lo_t = build_hist_nc._sent
                    nc.sync.dma_start(
                        out=hist[lo_t:done].rearrange(
                            "t (u w) -> u t w", u=V1),
                        in_=osb[:, lo_t * V2:done * V2].rearrange(
                            "u (t w) -> u t w", w=V2))
                    build_hist_nc._sent = done
    nc.compile()
    return nc


def build_fc_nc():
    """sim2 = xh @ xs^T [50,150]; win[t,l] = sim2[t, t+l]; out = relu(win@W^T + b)."""
    nc = bacc.Bacc("TRN2")
    # columns 0:50 = x_half^T, 50:200 = padded-context^T (one DMA -> one sem wait)
    # xn context packed [128, 4*200] bf16: block a, col t = component
    # a*128+p of frame t -> one contiguous 1600B-per-partition DMA
    xpk = nc.dram_tensor("xpk", [P, 4 * 200], BF16, kind="ExternalInput")
    wT = nc.dram_tensor("wT", [LW, P], F32R, kind="ExternalInput")
    out = nc.dram_tensor("out", [P, NF], F32, kind="ExternalOutput")
    # rows written at stride 163 (sim2[t] at 163*t), diagonal read back at
    # stride 164: addr 164*t + l = 163*t + (t+l) = sim2[t, t+l]  (no overlap)
    scratch = nc.dram_tensor("scratch", [NF * 164], F32, kind="Internal")

    with tile.TileContext(nc) as tc:
        with (
            tc.tile_pool(name="sb", bufs=1) as sb,
            tc.tile_pool(name="ps", bufs=1, space="PSUM") as ps,
        ):
            xa_sb = sb.tile([P, 4 * 200], BF16)
            nc.sync.dma_start(out=xa_sb[:], in_=xpk[:])
            wt_sb = sb.tile([LW, P], F32R)
            nc.sync.dma_start(out=wt_sb[:], in_=wT[:])

            sim_ps = ps.tile([NF, 150], F32)
            for a in range(4):
                nc.tensor.matmul(
                    out=sim_ps[:],
                    lhsT=xa_sb[:, a * 200:a * 200 + NF],
                    rhs=xa_sb[:, a * 200 + NF:(a + 1) * 200],
                    start=(a == 0), stop=(a == 3))
            sim_sb = sb.tile([NF, 150], F32)
            nc.vector.tensor_copy(out=sim_sb[:], in_=sim_ps[:])

            # row t of sim2 lands at flat offset 163*t
            nc.sync.dma_start(
                out=scratch[0:NF * 163].rearrange("(t c) -> t c", c=163)[:, 0:150],
                in_=sim_sb[:])
            # diagonal: win[t, l] = scratch[164*t + l] = sim2[t, t+l]
            win_sb = sb.tile([NF, LW], F32)
            nc.sync.dma_start(
                out=win_sb[:],
                in_=scratch[0:NF * 164].rearrange("(t c) -> t c", c=164)[:, 0:LW])

            # transpose win [50, 101] -> [101, 50] on the PE
            ident = sb.tile([NF, NF], F32)
            make_identity(nc, ident[:])
            win_ps = ps.tile([LW, NF], F32)
            nc.tensor.transpose(out=win_ps[:], in_=win_sb[:], identity=ident[:])
            win2 = sb.tile([LW, NF], F32R)
            nc.vector.tensor_copy(out=win2[:], in_=win_ps[:])

            fc_ps = ps.tile([P, NF], F32)
            nc.tensor.matmul(out=fc_ps[:], lhsT=wt_sb[:], rhs=win2[:],
                             start=True, stop=True)
            res = sb.tile([P, NF], F32)
            nc.vector.tensor_copy(out=res[:], in_=fc_ps[:])
            # bias + relu applied on host (tiny); avoids a 2-wait Activation
            nc.sync.dma_start(out=out[:], in_=res[:])
    nc.compile()
    return nc


_NC_CACHE = {}


def _get_nc(key, builder):
    if key not in _NC_CACHE:
        _NC_CACHE[key] = builder()
    return _NC_CACHE[key]


def kernel(frames, W, b):
    frames = np.asarray(frames, dtype=np.int32)
    W = np.asarray(W, dtype=np.float32)
    b = np.asarray(b, dtype=np.float32)
    Bn, _, T = frames.shape[:3]  # [4, 3, 100, 224, 224]

    nc_a = _get_nc("A", build_hist_nc)
    in_maps = []
    for c in range(NCORES):
        bi, h = c // 2, c % 2
        sl = frames[bi, :, h * NF:(h + 1) * NF].reshape(3, NF, NPIX)
        in_maps.append({"fr": np.ascontiguousarray(sl.astype(np.uint16))})
    res_a = run_bass_kernel_spmd(nc_a, in_maps, list(range(NCORES))).results

    counts = np.zeros((Bn, T, 512), np.float32)
    for c in range(NCORES):
        bi, h = c // 2, c % 2
        m = res_a[c]["hist"].astype(np.float64).reshape(NF, V1, V2)
        cts = np.einsum("ua,tuw,wb->tab", AINV.T, m, BINV.T)
        counts[bi, h * NF:(h + 1) * NF] = cts.reshape(NF, 512)
    xn = counts / np.linalg.norm(counts, axis=2, keepdims=True)

    nc_b = _get_nc("B", build_fc_nc)
    wT = np.ascontiguousarray(W.T)           # [101, 128]
    in_maps = []
    for c in range(NCORES):
        bi, h = c // 2, c % 2
        t0 = h * NF
        xall = np.zeros((200, 512), np.float32)
        xall[0:NF] = xn[bi, t0:t0 + NF]                  # x_half
        xall[NF + 50 - t0:NF + 50 - t0 + T] = xn[bi]     # xs[s'] = xn[s'+t0-50]
        xpk = (xall.T.reshape(4, P, 200).transpose(1, 0, 2)
               .reshape(P, 800).astype(ml_dtypes.bfloat16))
        in_maps.append({"xpk": np.ascontiguousarray(xpk), "wT": wT})
    res_b = run_bass_kernel_spmd(nc_b, in_maps, list(range(NCORES))).results

    outp = np.zeros((Bn, T, P), np.float32)
    for c in range(NCORES):
        bi, h = c // 2, c % 2
        outp[bi, h * NF:(h + 1) * NF] = res_b[c]["out"].T
    outp = np.maximum(outp + b[None, None, :], 0.0)
    return outp


# revision 35
# speedup vs baseline: 1.0048x; 1.0007x over previous
"""Trainium2 Bass kernel for nn_AutoShot (histogram binning + windowed similarity + FC).

Sharding: data-parallel over B*T = 400 frames -> 8 cores x 50 frames.

Phase A (heavy): per-core color histograms [50, 512].
  bin = hi5 * 16 + lo4 with hi5 = ((R>>3)&28)|(G>>6), lo4 = ((G>>2)&8)|(B>>5).
  Instead of one-hot factors, each side uses an arbitrary INVERTIBLE basis:
    M[i, j] = sum_px Abasis_i(hi_px) * Bbasis_j(lo_px)   (PE, PSUM-accumulated)
    counts  = Ainv @ M @ Binv^T                          (host, float64, exact)
  Basis columns are chosen by per-engine cost so all three elementwise
  engines run flat out in parallel:
    - free: ones (Pool memset), a2=(R>>3)&28, b2=G>>6, c2=(G>>2)&8, d2=B>>5
      (prep byproducts cast straight into the operand tiles)
    - DVE: is_equal (tensor_scalar, 4x perf mode, ~265ns/col)
    - ACT: exact sign staircases sgn(x - (k+.5)) (1 op/col, ~842ns)
    - Pool: is_equal tensor_scalar (~1186ns/col); one column is split
      DVE/Pool for load balance
  All products are small integers; fp32 PSUM sums stay < 2^24 -> exact.
  Channel planes are uint16 so every DVE op runs in the 4x perf mode.
  Batch schedule [1]+[2]*24+[1] frames shrinks pipeline fill/drain tails;
  finished frames stream to DRAM incrementally.

Phase B (light): per-core sim = xh @ xs^T (xs = zero-padded +-50 frame
  context) in float32r, diagonal window extract via a stride-164 read over
  stride-163 rows in a DRAM scratch (addr 164*t + l = sim[t, t+l]), PE
  transpose, FC matmul (W [128,101]), output written [P, NF]-major to keep
  the out-DMA contiguous.

Host: slices inputs, reconstructs counts (tiny 32x32/16x16 inverses),
  L2-normalizes, applies bias + ReLU, reassembles the [4,100,128] output.
"""

import sys

for _p in ("/opt/trn_rl_repo", "/root/.axon_site/_ro/trn_rl_repo"):
    if _p not in sys.path:
        sys.path.append(_p)

import numpy as np
import ml_dtypes

from concourse import bass, bacc, mybir
import concourse.tile as tile
from concourse.bass_utils import run_bass_kernel_spmd
from concourse.masks import make_identity

P = 128
NPIX = 224 * 224        # 50176 pixels per frame plane
FPP = NPIX // P         # 392 pixels per partition
NF = 50                 # frames per core
V1, V2 = 32, 16         # 512 = 32 * 16 bin split
LW = 101
NCORES = 8
F32 = mybir.dt.float32
F32R = mybir.dt.float32r
U16 = mybir.dt.uint16
BF16 = mybir.dt.bfloat16
OP = mybir.AluOpType
AF = mybir.ActivationFunctionType

# ---------------------------------------------------------------------------
# Basis column specs.  Each entry: ("ones"|"a2"|"b2"|"c2"|"d2") for free
# columns, ("dve", v) / ("pool", v) for is_equal(v), ("act", thr) for
# sgn(x - thr).  The same spec drives the device builder and the host-side
# inverse (evaluated in numpy below).
USE_POOL = True
USE_ACT = True


def _mk_specs():
    # A side: free rows resolve points {0,3,4}; deltas at [1,2]+[5..23]
    # (17 DVE + 4 Pool); ACT staircases 23.5..30.5 resolve 24..31.
    a_spec = [("ones",), ("a2",), ("b2",)]
    if USE_ACT:
        a_delta = [1, 2] + list(range(5, 24))
        act_lo = 23
    else:
        a_delta = [1, 2] + list(range(5, 32))
        act_lo = 31
    n_pool_a = 4 if USE_POOL else 0
    n_dve_a = len(a_delta) - n_pool_a
    for i, p in enumerate(a_delta[:n_dve_a]):
        kind = "dvepool" if (USE_POOL and i == 0) else "dve"
        a_spec.append((kind, float(p)))
    for p in a_delta[len(a_delta) - n_pool_a:]:
        a_spec.append(("pool", float(p)))
    for t in range(act_lo, 31):
        a_spec.append(("act", t + 0.5))
    # B side: free rows resolve {0,7,8}; deltas at [1..6, 9..12]
    # (8 DVE + 2 Pool); ACT staircases 12.5/13.5/14.5 resolve 13..15.
    b_spec = [("ones",), ("c2",), ("d2",)]
    b_delta = [1, 2, 3, 4, 5, 6, 9, 10, 11, 12]
    n_pool_b = 2 if USE_POOL else 0
    for p in b_delta[: len(b_delta) - n_pool_b]:
        b_spec.append(("dve", float(p)))
    for p in b_delta[len(b_delta) - n_pool_b:]:
        b_spec.append(("pool", float(p)))
    if USE_ACT:
        for t in range(12, 15):
            b_spec.append(("act", t + 0.5))
    else:
        for p in range(13, 16):
            b_spec.append(("dve", float(p)))
    assert len(a_spec) == V1 and len(b_spec) == V2
    return a_spec, b_spec


A_SPEC, B_SPEC = _mk_specs()


def _basis_matrix(spec, nvals, a_side):
    """Rows = basis columns, cols = alphabet values; float64."""
    v = np.arange(nvals, dtype=np.float64)
    rows = []
    for s in spec:
        kind = s[0]
        if kind == "ones":
            rows.append(np.ones(nvals))
        elif kind == "a2":
            rows.append(np.float64(np.arange(nvals) & 28))
        elif kind == "b2":
            rows.append(np.float64(np.arange(nvals) & 3))
        elif kind == "c2":
            rows.append(np.float64(np.arange(nvals) & 8))
        elif kind == "d2":
            rows.append(np.float64(np.arange(nvals) & 7))
        elif kind in ("dve", "pool", "dvepool"):
            rows.append((v == s[1]).astype(np.float64))
        elif kind == "act":
            rows.append(np.sign(v - s[1]))
        else:
            raise ValueError(kind)
    return np.stack(rows)


ABASIS = _basis_matrix(A_SPEC, V1, True)
BBASIS = _basis_matrix(B_SPEC, V2, False)
AINV = np.linalg.inv(ABASIS)
BINV = np.linalg.inv(BBASIS)
assert np.linalg.cond(ABASIS) < 1e4 and np.linalg.cond(BBASIS) < 1e4


def build_hist_nc():
    nc = bacc.Bacc("TRN2")
    fr = nc.dram_tensor("fr", [3, NF, NPIX], U16, kind="ExternalInput")
    hist = nc.dram_tensor("hist", [NF, 512], F32, kind="ExternalOutput")
    G = 2                # frames per DVE batch
    FD = G * FPP         # 784 free-dim elements per DVE op

    act_bias = {}        # threshold -> AP [P,1] holding -thr

    def emit_col(tile_, idx, fd, src_tile, s):
        kind = s[0]
        lo = idx * fd
        if kind == "dvepool":
            h = (fd // 2) // 4 * 4
            nc.vector.tensor_scalar(
                out=tile_[:, lo:lo + h], in0=src_tile[:, 0:h], scalar1=s[1],
                scalar2=None, op0=OP.is_equal)
            nc.gpsimd.tensor_scalar(
                out=tile_[:, lo + h:lo + fd], in0=src_tile[:, h:fd],
                scalar1=s[1], scalar2=None, op0=OP.is_equal)
        elif kind == "dve":
            nc.vector.tensor_scalar(
                out=tile_[:, lo:lo + fd], in0=src_tile[:, 0:fd], scalar1=s[1],
                scalar2=None, op0=OP.is_equal)
        elif kind == "pool":
            nc.gpsimd.tensor_scalar(
                out=tile_[:, lo:lo + fd], in0=src_tile[:, 0:fd], scalar1=s[1],
                scalar2=None, op0=OP.is_equal)
        elif kind == "act":
            nc.scalar.activation(out=tile_[:, lo:lo + fd],
                                 in_=src_tile[:, 0:fd], func=AF.Sign,
                                 bias=act_bias[s[1]])
        elif kind == "ones":
            nc.gpsimd.memset(tile_[:, lo:lo + fd], 1.0)
        else:
            raise ValueError(kind)

    with tile.TileContext(nc) as tc:
        with (
            tc.tile_pool(name="io", bufs=3) as io,
            tc.tile_pool(name="mid", bufs=2) as mid,
            tc.tile_pool(name="oh", bufs=2) as oh,
            tc.tile_pool(name="cst", bufs=1) as cst,
            tc.tile_pool(name="ps", bufs=8, space="PSUM") as ps,
        ):
            osb = cst.tile([V1, NF * V2], F32)  # [32, 800] result staging

            thrs = sorted({s[1] for s in A_SPEC + B_SPEC if s[0] == "act"})
            bias_sb = cst.tile([P, max(len(thrs), 1)], F32)
            for i, thr in enumerate(thrs):
                nc.vector.memset(bias_sb[:, i:i + 1], -float(thr))
                act_bias[thr] = bias_sb[:, i:i + 1]

            # batch schedule: small first/last batches shrink pipeline
            # fill (PE idle) and drain (featurize engines idle) tails
            batches = []
            t = 0
            for gsz in [1] + [2] * ((NF - 2) // 2) + [1]:
                batches.append((t, gsz))
                t += gsz
            assert t == NF

            ia2 = A_SPEC.index(("a2",))
            ib2 = A_SPEC.index(("b2",))
            ic2 = B_SPEC.index(("c2",))
            id2 = B_SPEC.index(("d2",))

            build_hist_nc._sent = 0
            pending = []
            for t0, gsz in batches:
                fd = gsz * FPP
                r = io.tile([P, fd], U16, tag="ch_r")
                g = io.tile([P, fd], U16, tag="ch_g")
                b = io.tile([P, fd], U16, tag="ch_b")
                for ci, ch in ((0, r), (1, g), (2, b)):
                    nc.sync.dma_start(
                        out=ch[:].rearrange("p (q f) -> p q f", q=gsz),
                        in_=fr[ci, t0:t0 + gsz].rearrange(
                            "q (p f) -> p q f", p=P))

                A = oh.tile([P, V1 * fd], BF16, tag="A")
                B = oh.tile([P, V2 * fd], BF16, tag="B")

                def col(tile_, idx):
                    return tile_[:, idx * fd:(idx + 1) * fd]

                # prep: bitvec ops stay u16->u16, then numeric casts (4x)
                # write the byproducts into the operand tiles as bf16
                a2u = mid.tile([P, fd], U16, tag="a2u")
                nc.vector.tensor_scalar(
                    out=a2u[:], in0=r[:], scalar1=3, scalar2=28,
                    op0=OP.logical_shift_right, op1=OP.bitwise_and)
                b2u = mid.tile([P, fd], U16, tag="b2u")
                nc.vector.tensor_scalar(
                    out=b2u[:], in0=g[:], scalar1=6, scalar2=None,
                    op0=OP.logical_shift_right)
                nc.vector.tensor_copy(out=col(A, ia2), in_=a2u[:])
                nc.vector.tensor_copy(out=col(A, ib2), in_=b2u[:])
                hi_b = mid.tile([P, fd], BF16, tag="hi")
                nc.vector.tensor_tensor(
                    out=hi_b[:], in0=col(A, ia2), in1=col(A, ib2), op=OP.add)
                c2u = mid.tile([P, fd], U16, tag="c2u")
                nc.vector.tensor_scalar(
                    out=c2u[:], in0=g[:], scalar1=2, scalar2=8,
                    op0=OP.logical_shift_right, op1=OP.bitwise_and)
                d2u = mid.tile([P, fd], U16, tag="d2u")
                nc.vector.tensor_scalar(
                    out=d2u[:], in0=b[:], scalar1=5, scalar2=None,
                    op0=OP.logical_shift_right)
                nc.vector.tensor_copy(out=col(B, ic2), in_=c2u[:])
                nc.vector.tensor_copy(out=col(B, id2), in_=d2u[:])
                lo_b = mid.tile([P, fd], BF16, tag="lo")
                nc.vector.tensor_tensor(
                    out=lo_b[:], in0=col(B, ic2), in1=col(B, id2), op=OP.add)

                # non-DVE columns first so ACT/Pool queues fill early
                order = sorted(
                    range(V1), key=lambda i: A_SPEC[i][0] in ("dve", "dvepool"))
                for i in order:
                    s = A_SPEC[i]
                    if s[0] in ("a2", "b2"):
                        continue
                    emit_col(A, i, fd, hi_b, s)
                order = sorted(
                    range(V2), key=lambda i: B_SPEC[i][0] in ("dve", "dvepool"))
                for i in order:
                    s = B_SPEC[i]
                    if s[0] in ("c2", "d2"):
                        continue
                    emit_col(B, i, fd, lo_b, s)

                # contract over pixels per frame: M[i, j] += A_qj^T @ B_qj
                Aq = A[:].rearrange("p (v q f) -> p q f v", v=V1, q=gsz)
                Bq = B[:].rearrange("p (v q f) -> p q f v", v=V2, q=gsz)
                for q in range(gsz):
                    hps = ps.tile([V1, V2], F32)
                    for j in range(FPP):
                        nc.tensor.matmul(
                            out=hps[:],
                            lhsT=Aq[:, q, j, :],
                            rhs=Bq[:, q, j, :],
                            start=(j == 0), stop=(j == FPP - 1))
                    pending.append((t0 + q, hps))

                # drain PSUM two batches late so the in-order ACT queue
                # never blocks on a still-running PE accumulation, then
                # stream finished frames out incrementally
                flush = len(pending) if t0 + gsz == NF else max(
                    0, len(pending) - 4)
                for _ in range(flush):
                    t, hp = pending.pop(0)
                    nc.scalar.activation(
                        out=osb[:, t * V2:(t + 1) * V2], in_=hp[:],
                        func=AF.Copy)
                    drained = t + 1
                done = drained if flush else 0
                if done and (done - build_hist_nc._sent >= 8 or done == NF):
                    lo_t = build_hist_nc._sent
                    nc.sync.dma_start(
                        out=hist[lo_t:done].rearrange(
                            "t (u w) -> u t w", u=V1),
                        in_=osb[:, lo_t * V2:done * V2].rearrange(
                            "u (t w) -> u t w", w=V2))
                    build_hist_nc._sent = done
    nc.compile()
    return nc


def build_fc_nc():
    """sim2 = xh @ xs^T [50,150]; win[t,l] = sim2[t, t+l]; out = relu(win@W^T + b)."""
    nc = bacc.Bacc("TRN2")
    # columns 0:50 = x_half^T, 50:200 = padded-context^T (one DMA -> one sem wait)
    # xn context packed [128, 4*200] bf16: block a, col t = component
    # a*128+p of frame t -> one contiguous 1600B-per-partition DMA
    xpk = nc.dram_tensor("xpk", [P, 4 * 200], BF16, kind="ExternalInput")
    wT = nc.dram_tensor("wT", [LW, P], F32R, kind="ExternalInput")
    out = nc.dram_tensor("out", [P, NF], F32, kind="ExternalOutput")
    # rows written at stride 163 (sim2[t] at 163*t), diagonal read back at
    # stride 164: addr 164*t + l = 163*t + (t+l) = sim2[t, t+l]  (no overlap)
    scratch = nc.dram_tensor("scratch", [NF * 164], F32, kind="Internal")

    with tile.TileContext(nc) as tc:
        with (
            tc.tile_pool(name="sb", bufs=1) as sb,
            tc.tile_pool(name="ps", bufs=1, space="PSUM") as ps,
        ):
            xa_sb = sb.tile([P, 4 * 200], BF16)
            nc.sync.dma_start(out=xa_sb[:], in_=xpk[:])
            wt_sb = sb.tile([LW, P], F32R)
            nc.sync.dma_start(out=wt_sb[:], in_=wT[:])

            sim_ps = ps.tile([NF, 150], F32)
            for a in range(4):
                nc.tensor.matmul(
                    out=sim_ps[:],
                    lhsT=xa_sb[:, a * 200:a * 200 + NF],
                    rhs=xa_sb[:, a * 200 + NF:(a + 1) * 200],
                    start=(a == 0), stop=(a == 3))
            sim_sb = sb.tile([NF, 150], F32)
            nc.vector.tensor_copy(out=sim_sb[:], in_=sim_ps[:])

            # row t of sim2 lands at flat offset 163*t
            nc.sync.dma_start(
                out=scratch[0:NF * 163].rearrange("(t c) -> t c", c=163)[:, 0:150],
                in_=sim_sb[:])
            # diagonal: win[t, l] = scratch[164*t + l] = sim2[t, t+l]
            win_sb = sb.tile([NF, LW], F32)
            nc.sync.dma_start(
                out=win_sb[:],
                in_=scratch[0:NF * 164].rearrange("(t c) -> t c", c=164)[:, 0:LW])

            # transpose win [50, 101] -> [101, 50] on the PE
            ident = sb.tile([NF, NF], F32)
            make_identity(nc, ident[:])
            win_ps = ps.tile([LW, NF], F32)
            nc.tensor.transpose(out=win_ps[:], in_=win_sb[:], identity=ident[:])
            win2 = sb.tile([LW, NF], F32R)
            nc.vector.tensor_copy(out=win2[:], in_=win_ps[:])

            fc_ps = ps.tile([P, NF], F32)
            nc.tensor.matmul(out=fc_ps[:], lhsT=wt_sb[:], rhs=win2[:],
                             start=True, stop=True)
            res = sb.tile([P, NF], F32)
            nc.vector.tensor_copy(out=res[:], in_=fc_ps[:])
            # bias + relu applied on host (tiny); avoids a 2-wait Activation
            nc.sync.dma_start(out=out[:], in_=res[:])
    nc.compile()
    return nc


_NC_CACHE = {}


def _get_nc(key, builder):
    if key not in _NC_CACHE:
        _NC_CACHE[key] = builder()
    return _NC_CACHE[key]


def kernel(frames, W, b):
    frames = np.asarray(frames, dtype=np.int32)
    W = np.asarray(W, dtype=np.float32)
    b = np.asarray(b, dtype=np.float32)
    Bn, _, T = frames.shape[:3]  # [4, 3, 100, 224, 224]

    nc_a = _get_nc("A", build_hist_nc)
    in_maps = []
    for c in range(NCORES):
        bi, h = c // 2, c % 2
        sl = frames[bi, :, h * NF:(h + 1) * NF].reshape(3, NF, NPIX)
        in_maps.append({"fr": np.ascontiguousarray(sl.astype(np.uint16))})
    res_a = run_bass_kernel_spmd(nc_a, in_maps, list(range(NCORES))).results

    counts = np.zeros((Bn, T, 512), np.float32)
    for c in range(NCORES):
        bi, h = c // 2, c % 2
        m = res_a[c]["hist"].astype(np.float64).reshape(NF, V1, V2)
        cts = np.einsum("ua,tuw,wb->tab", AINV.T, m, BINV.T)
        counts[bi, h * NF:(h + 1) * NF] = cts.reshape(NF, 512)
    xn = counts / np.linalg.norm(counts, axis=2, keepdims=True)

    nc_b = _get_nc("B", build_fc_nc)
    wT = np.ascontiguousarray(W.T)           # [101, 128]
    in_maps = []
    for c in range(NCORES):
        bi, h = c // 2, c % 2
        t0 = h * NF
        xall = np.zeros((200, 512), np.float32)
        xall[0:NF] = xn[bi, t0:t0 + NF]                  # x_half
        xall[NF + 50 - t0:NF + 50 - t0 + T] = xn[bi]     # xs[s'] = xn[s'+t0-50]
        xpk = (xall.T.reshape(4, P, 200).transpose(1, 0, 2)
               .reshape(P, 800).astype(ml_dtypes.bfloat16))
        in_maps.append({"xpk": np.ascontiguousarray(xpk), "wT": wT})
    res_b = run_bass_kernel_spmd(nc_b, in_maps, list(range(NCORES))).results

    outp = np.zeros((Bn, T, P), np.float32)
    for c in range(NCORES):
        bi, h = c // 2, c % 2
        outp[bi, h * NF:(h + 1) * NF] = res_b[c]["out"].T
    outp = np.maximum(outp + b[None, None, :], 0.0)
    return outp
